# revision 1
# baseline (speedup 1.0000x reference)
"""Causal self-attention (GPT-style) Trainium2 Bass kernel.

Problem: x[2,4096,768] -> qkv = x@W_attn+b_attn -> 12-head causal attention
-> out @ W_proj + b_proj.   B=2, T=4096, C=768, H=12, Dh=64.

Sharding: (batch, head) parallel over 8 cores. Core c handles batch c//4 and
heads 3*(c%4) .. 3*(c%4)+2.  Each core computes qkv for its 3 heads, causal
attention, and a partial output projection (rows of W_proj for its heads).
Host sums the 4 partials per batch (tensor-parallel reduce) and transposes.

On-device layout choices:
- Scores are computed transposed: S^T[t_k, q] = K^T_tile.T @ Q^T so the
  exp output P^T feeds the PV matmul directly (no on-chip transposes).
- Softmax denominator Z comes for free from an appended ones-column on V
  (PV matmul row 64 = column sums of P^T).
- Softmax max-subtraction is skipped: scores are ~N(0,1) for this problem
  (max |s| < ~8), far from fp32/bf16 overflow.
- Matmul inputs are bf16 (host-precast), accumulation fp32.
- Causal masking: only lower-left k-tiles are computed; the 4 diagonal
  k-tiles per q-chunk get a 128x128 triangular band mask multiplied into
  P^T on GPSIMD, plus a restricted PV read range.
"""

import math
import sys

sys.path.insert(0, "/opt/trn_rl_repo")

import numpy as np
import ml_dtypes

B, T, C = 2, 4096, 768
NH, DH = 12, 64
HPC = 3          # heads per core
NCORES = 8
NQ = T // 512    # q chunks (8)
NKT = T // 128   # k tiles (32)

BF16 = ml_dtypes.bfloat16

_PROG = None


def _build_program():
    import concourse.bass as bass
    import concourse.mybir as mybir
    import concourse.tile as tile
    from concourse import bacc

    f32 = mybir.dt.float32
    bf16 = mybir.dt.bfloat16
    Exp = mybir.ActivationFunctionType.Exp
    mult = mybir.AluOpType.mult

    nc = bacc.Bacc("TRN2", target_bir_lowering=False)

    # ---- I/O ----
    xt_d = nc.dram_tensor("xt", [C, T], bf16, kind="ExternalInput")       # x[b].T
    wqk_d = nc.dram_tensor("wqk", [C, 384], bf16, kind="ExternalInput")   # [Q0|Q1|K0|K1|Q2|K2]
    wqkb_d = nc.dram_tensor("wqkb", [1, 384], bf16, kind="ExternalInput")
    wv_d = nc.dram_tensor("wv", [C, 195], bf16, kind="ExternalInput")     # per head [Wv|0]
    wvb_d = nc.dram_tensor("wvb", [1, 195], bf16, kind="ExternalInput")   # per head [b_v|1]
    wp_d = nc.dram_tensor("wp", [193, 768], bf16, kind="ExternalInput")   # rows + bias/4 row
    mask_d = nc.dram_tensor("mask", [128, 128], bf16, kind="ExternalInput")  # triu ones
    yt_d = nc.dram_tensor("yt", [C, T], f32, kind="ExternalOutput")       # y[b].T partial

    with tile.TileContext(nc) as tc:
        with (
            tc.tile_pool(name="const", bufs=1) as const,
            tc.tile_pool(name="big", bufs=1) as big,
        ):
            # persistent SBUF tensors
            xt_sb = big.tile([128, 6, T], bf16)         # x[b].T tiled over channel
            ones_sb = const.tile([1, T], bf16)
            wqk_sb = const.tile([128, 6, 384], bf16)
            wqkb_sb = const.tile([1, 384], bf16)
            wv_sb = const.tile([128, 6, 195], bf16)
            wvb_sb = const.tile([1, 195], bf16)
            wp0_sb = const.tile([128, 768], bf16)
            wp1_sb = const.tile([65, 768], bf16)
            mask_sb = const.tile([128, 128], bf16)
            qk_sb = big.tile([128, 3, T], bf16)         # [Q0|Q1],[K0|K1],[Q2|K2]
            kq2_sb = big.tile([128, T], bf16)           # [K2 @0:64 | Q2 @64:128]
            v_sb = big.tile([128, NKT, 195], bf16)      # v natural, per-head 65 cols
            attu_sb = [big.tile([65, T], f32, name=f"attu{h}") for h in range(HPC)]  # unnormalized attT + Z
            attn0_sb = big.tile([128, T], bf16)         # normalized attT h0|h1
            attn1_sb = big.tile([65, T], bf16)          # h2 | ones row

            nc.vector.memset(ones_sb, 1.0)
            nc.vector.memset(attn1_sb[64:65, :], 1.0)

            # ---- loads ----
            for k in range(6):
                for hhalf in range(2):
                    sl = slice(hhalf * (T // 2), (hhalf + 1) * (T // 2))
                    nc.sync.dma_start(
                        out=xt_sb[:, k, sl], in_=xt_d[k * 128:(k + 1) * 128, sl]
                    )
                nc.sync.dma_start(out=wqk_sb[:, k, :], in_=wqk_d[k * 128:(k + 1) * 128, :])
                nc.sync.dma_start(out=wv_sb[:, k, :], in_=wv_d[k * 128:(k + 1) * 128, :])
            nc.sync.dma_start(out=wqkb_sb, in_=wqkb_d[:, :])
            nc.sync.dma_start(out=wvb_sb, in_=wvb_d[:, :])
            nc.sync.dma_start(out=wp0_sb, in_=wp_d[0:128, :])
            nc.sync.dma_start(out=wp1_sb, in_=wp_d[128:193, :])
            nc.sync.dma_start(out=mask_sb, in_=mask_d[:, :])

            # ---- phase 1a: qkT = (wqk.T @ xt) + bias   -> [384, T] bf16 ----
            with tc.tile_pool(name="psq", bufs=2, space="PSUM") as psq:
                for n in range(NQ):
                    nsl = slice(n * 512, (n + 1) * 512)
                    for m in range(3):
                        msl = slice(m * 128, (m + 1) * 128)
                        ps = psq.tile([128, 512], f32)
                        for k in range(6):
                            nc.tensor.matmul(
                                ps,
                                lhsT=wqk_sb[:, k, msl],
                                rhs=xt_sb[:, k, nsl],
                                start=(k == 0),
                                stop=False,
                            )
                        nc.tensor.matmul(
                            ps,
                            lhsT=wqkb_sb[:, msl],
                            rhs=ones_sb[:, nsl],
                            start=False,
                            stop=True,
                        )
                        if (n * 3 + m) % 2 == 0:
                            nc.scalar.copy(out=qk_sb[:, m, nsl], in_=ps)
                        else:
                            nc.vector.tensor_copy(qk_sb[:, m, nsl], ps)

            # ---- phase 1b: v natural = xt.T @ wv + bias -> [T, 195] bf16 ----
            with tc.tile_pool(name="psv", bufs=2, space="PSUM") as psv:
                for tt in range(NKT):
                    tsl = slice(tt * 128, (tt + 1) * 128)
                    ps = psv.tile([128, 195], f32)
                    for k in range(6):
                        nc.tensor.matmul(
                            ps,
                            lhsT=xt_sb[:, k, tsl],
                            rhs=wv_sb[:, k, :],
                            start=(k == 0),
                            stop=False,
                        )
                    nc.tensor.matmul(
                        ps, lhsT=ones_sb[:, tsl], rhs=wvb_sb, start=False, stop=True
                    )
                    if tt % 2 == 0:
                        nc.vector.tensor_copy(v_sb[:, tt, :], ps)
                    else:
                        nc.scalar.copy(out=v_sb[:, tt, :], in_=ps)

            # h2 duplicates so consecutive h2 score matmuls alternate row-groups
            nc.sync.dma_start(out=kq2_sb[0:64, :], in_=qk_sb[64:128, 2, :])   # K2 -> base 0
            nc.sync.dma_start(out=kq2_sb[64:128, :], in_=qk_sb[0:64, 2, :])   # Q2 -> base 64

            def K_ap(h, kt):
                sl = slice(kt * 128, (kt + 1) * 128)
                if h == 0:
                    return qk_sb[0:64, 1, sl]
                if h == 1:
                    return qk_sb[64:128, 1, sl]
                return qk_sb[64:128, 2, sl] if kt % 2 == 0 else kq2_sb[0:64, sl]

            def Q_ap(h, qi, kt):
                sl = slice(qi * 512, (qi + 1) * 512)
                if h == 0:
                    return qk_sb[0:64, 0, sl]
                if h == 1:
                    return qk_sb[64:128, 0, sl]
                return kq2_sb[64:128, sl] if kt % 2 == 0 else qk_sb[0:64, 2, sl]

            # ---- phase 2: attention ----
            with (
                tc.tile_pool(name="sc", bufs=2, space="PSUM") as pssc,
                tc.tile_pool(name="at", bufs=1, space="PSUM") as psat,
                tc.tile_pool(name="pt", bufs=4) as ppt,
            ):
                for qi in range(NQ):
                    att = [
                        psat.tile([65, 512], f32, name=f"att{h}", tag=f"att{h}",
                                  bufs=2 if h == 0 else 1)
                        for h in range(HPC)
                    ]
                    n_kt = 4 * (qi + 1)
                    for kp in range(n_kt // 2):
                        kA, kB = 2 * kp, 2 * kp + 1
                        groups = [
                            [(0, kA), (1, kA)],
                            [(0, kB), (1, kB)],
                            [(2, kA), (2, kB)],
                        ]
                        for grp in groups:
                            sc = pssc.tile([128, 1024], f32)
                            for idx, (h, kt) in enumerate(grp):
                                nc.tensor.matmul(
                                    sc[:, idx * 512:(idx + 1) * 512],
                                    lhsT=K_ap(h, kt),
                                    rhs=Q_ap(h, qi, kt),
                                    start=True,
                                    stop=True,
                                )
                            pt = ppt.tile([128, 1024], bf16)
                            nc.scalar.activation(pt, sc, Exp, scale=1.0 / math.sqrt(DH))
                            for idx, (h, kt) in enumerate(grp):
                                j = kt - 4 * qi
                                if j >= 0:  # diagonal tile: band mask + restricted read
                                    band = pt[:, idx * 512 + 128 * j: idx * 512 + 128 * (j + 1)]
                                    nc.vector.tensor_tensor(band, band, mask_sb, mult)
                                    rhs = pt[:, idx * 512 + 128 * j:(idx + 1) * 512]
                                    outap = att[h][:, 128 * j:512]
                                else:
                                    rhs = pt[:, idx * 512:(idx + 1) * 512]
                                    outap = att[h][:, :]
                                nc.tensor.matmul(
                                    outap,
                                    lhsT=v_sb[:, kt, h * 65:(h + 1) * 65],
                                    rhs=rhs,
                                    start=(kt == 0),
                                    stop=(kt == n_kt - 1),
                                    skip_group_check=True,
                                )
                    for h in range(HPC):
                        nc.vector.tensor_copy(
                            attu_sb[h][:, qi * 512:(qi + 1) * 512], att[h]
                        )

            # ---- phase 3: normalize  attT_norm = attT_unnorm * (1/Z) ----
            with (
                tc.tile_pool(name="zdram", bufs=2, space="DRAM") as zdram,
                tc.tile_pool(name="zsmall", bufs=4) as zsmall,
                tc.tile_pool(name="rbp", bufs=1) as rbp,
            ):
                for h in range(HPC):
                    zbuf = zdram.tile([1, T], f32)
                    rbuf = zdram.tile([1, T], f32)
                    nc.sync.dma_start(out=zbuf[:, :], in_=attu_sb[h][64:65, :])
                    zt = zsmall.tile([128, T // 128], f32)
                    nc.sync.dma_start(
                        out=zt, in_=zbuf.rearrange("p (a b) -> (p a) b", a=128)
                    )
                    rz = zsmall.tile([128, T // 128], f32)
                    nc.vector.reciprocal(rz, zt)
                    nc.sync.dma_start(
                        out=rbuf.rearrange("p (a b) -> (p a) b", a=128), in_=rz
                    )
                    rb = rbp.tile([64, T], f32)
                    nc.sync.dma_start(
                        out=rb,
                        in_=bass.AP(tensor=rbuf.tensor, offset=rbuf.offset, ap=[[0, 64], [1, T]]),
                    )
                    dst = (
                        attn0_sb[0:64, :]
                        if h == 0
                        else (attn0_sb[64:128, :] if h == 1 else attn1_sb[0:64, :])
                    )
                    nc.vector.tensor_tensor(dst, attu_sb[h][0:64, :], rb, mult)

            # ---- phase 4: y^T partial = wp.T @ attT_norm (+ b/4 via ones row) ----
            with (
                tc.tile_pool(name="psp", bufs=3, space="PSUM") as psp,
                tc.tile_pool(name="yst", bufs=4) as yst,
            ):
                for n in range(NQ):
                    nsl = slice(n * 512, (n + 1) * 512)
                    for m in range(6):
                        msl = slice(m * 128, (m + 1) * 128)
                        ps = psp.tile([128, 512], f32)
                        nc.tensor.matmul(
                            ps, lhsT=wp0_sb[:, msl], rhs=attn0_sb[:, nsl],
                            start=True, stop=False,
                        )
                        nc.tensor.matmul(
                            ps, lhsT=wp1_sb[:, msl], rhs=attn1_sb[:, nsl],
                            start=False, stop=True,
                        )
                        ysb = yst.tile([128, 512], f32)
                        if m % 2 == 0:
                            nc.vector.tensor_copy(ysb, ps)
                        else:
                            nc.scalar.copy(out=ysb, in_=ps)
                        nc.sync.dma_start(out=yt_d[msl, nsl], in_=ysb)

    nc.finalize()
    return nc


def _get_program():
    global _PROG
    if _PROG is None:
        _PROG = _build_program()
    return _PROG


def _core_inputs(x, W_attn, b_attn, W_proj, b_proj, core):
    b = core // 4
    h0 = HPC * (core % 4)

    def qcol(h):
        return W_attn[:, h * 64:(h + 1) * 64]

    def kcol(h):
        return W_attn[:, C + h * 64:C + (h + 1) * 64]

    def vcol(h):
        return W_attn[:, 2 * C + h * 64:2 * C + (h + 1) * 64]

    def qb(h):
        return b_attn[h * 64:(h + 1) * 64]

    def kb(h):
        return b_attn[C + h * 64:C + (h + 1) * 64]

    def vb(h):
        return b_attn[2 * C + h * 64:2 * C + (h + 1) * 64]

    xt = np.ascontiguousarray(x[b].T).astype(BF16)
    wqk = np.concatenate(
        [qcol(h0), qcol(h0 + 1), kcol(h0), kcol(h0 + 1), qcol(h0 + 2), kcol(h0 + 2)],
        axis=1,
    ).astype(BF16)
    wqkb = np.concatenate(
        [qb(h0), qb(h0 + 1), kb(h0), kb(h0 + 1), qb(h0 + 2), kb(h0 + 2)]
    )[None, :].astype(BF16)
    zcol = np.zeros((C, 1), np.float32)
    wv = np.concatenate(
        [vcol(h0), zcol, vcol(h0 + 1), zcol, vcol(h0 + 2), zcol], axis=1
    ).astype(BF16)
    one = np.ones((1,), np.float32)
    wvb = np.concatenate([vb(h0), one, vb(h0 + 1), one, vb(h0 + 2), one])[None, :].astype(BF16)
    wp = np.concatenate(
        [W_proj[h0 * 64:(h0 + HPC) * 64, :], (b_proj / 4.0)[None, :]], axis=0
    ).astype(BF16)
    mask = np.triu(np.ones((128, 128), np.float32)).astype(BF16)
    return {
        "xt": xt, "wqk": wqk, "wqkb": wqkb, "wv": wv, "wvb": wvb,
        "wp": wp, "mask": mask,
    }


def _run(x, W_attn, b_attn, W_proj, b_proj, trace=False):
    from concourse.bass_utils import run_bass_kernel_spmd

    x = np.asarray(x, dtype=np.float32)
    W_attn = np.asarray(W_attn, dtype=np.float32)
    b_attn = np.asarray(b_attn, dtype=np.float32)
    W_proj = np.asarray(W_proj, dtype=np.float32)
    b_proj = np.asarray(b_proj, dtype=np.float32)

    nc = _get_program()
    in_maps = [
        _core_inputs(x, W_attn, b_attn, W_proj, b_proj, c) for c in range(NCORES)
    ]
    res = run_bass_kernel_spmd(
        nc, in_maps, core_ids=list(range(NCORES)), trace=trace
    )
    y = np.zeros((B, T, C), np.float32)
    for c in range(NCORES):
        y[c // 4] += res.results[c]["yt"].T
    return y, res


def kernel(x, W_attn, b_attn, W_proj, b_proj):
    y, _ = _run(x, W_attn, b_attn, W_proj, b_proj)
    return y



# revision 22
# speedup vs baseline: 1.0745x; 1.0745x over previous
"""Causal self-attention (GPT-style) Trainium2 Bass kernel.

Problem: x[2,4096,768] -> qkv = x@W_attn+b_attn -> 12-head causal attention
-> out @ W_proj + b_proj.   B=2, T=4096, C=768, H=12, Dh=64.

Sharding: (batch, head) parallel over 8 cores. Core c handles batch c//4 and
heads 3*(c%4) .. 3*(c%4)+2.  Each core computes qkv for its 3 heads, causal
attention, and a partial output projection (rows of W_proj for its heads).
Host sums the 4 partials per batch (tensor-parallel reduce) and transposes.

On-device layout choices:
- Scores are computed transposed: S^T[t_k, q] = K^T_tile.T @ Q^T so the
  exp output P^T feeds the PV matmul directly (no on-chip transposes).
- Softmax denominator Z comes for free from an appended ones-column on V
  (PV matmul row 64 = column sums of P^T).
- Softmax max-subtraction is skipped: scores are ~N(0,1) for this problem
  (max |s| < ~8), far from fp32/bf16 overflow.
- Matmul inputs are bf16 (host-precast), accumulation fp32.
- Causal masking: only lower-left k-tiles are computed; the 4 diagonal
  k-tiles per q-chunk get a 128x128 triangular band mask multiplied into
  P^T on GPSIMD, plus a restricted PV read range.
"""

import math
import sys

sys.path.insert(0, "/opt/trn_rl_repo")

import numpy as np
import ml_dtypes

B, T, C = 2, 4096, 768
NH, DH = 12, 64
HPC = 3          # heads per core
NCORES = 8
NQ = T // 512    # q chunks (8)
NKT = T // 128   # k tiles (32)

BF16 = ml_dtypes.bfloat16

_PROG = None


def _build_program():
    import concourse.bass as bass
    import concourse.mybir as mybir
    import concourse.tile as tile
    from concourse import bacc

    f32 = mybir.dt.float32
    bf16 = mybir.dt.bfloat16
    Exp = mybir.ActivationFunctionType.Exp
    mult = mybir.AluOpType.mult

    nc = bacc.Bacc("TRN2", target_bir_lowering=False)

    # ---- I/O ----
    xt_d = nc.dram_tensor("xt", [C, T], bf16, kind="ExternalInput")       # x[b].T
    wqk_d = nc.dram_tensor("wqk", [C, 384], bf16, kind="ExternalInput")   # [Q0|Q1|K0|K1|Q2|K2]
    wqkb_d = nc.dram_tensor("wqkb", [1, 384], bf16, kind="ExternalInput")
    wv_d = nc.dram_tensor("wv", [C, 195], bf16, kind="ExternalInput")     # per head [Wv|0]
    wvb_d = nc.dram_tensor("wvb", [1, 195], bf16, kind="ExternalInput")   # per head [b_v|1]
    wp_d = nc.dram_tensor("wp", [193, 768], bf16, kind="ExternalInput")   # rows + bias/4 row
    mask_d = nc.dram_tensor("mask", [128, 128], bf16, kind="ExternalInput")  # triu ones
    yt_d = nc.dram_tensor("yt", [C, T], bf16, kind="ExternalOutput")      # y[b].T partial

    with tile.TileContext(nc) as tc:
        with (
            tc.tile_pool(name="const", bufs=1) as const,
            tc.tile_pool(name="big", bufs=1) as big,
        ):
            # persistent SBUF tensors
            xt_sb = big.tile([128, 6, T], bf16)         # x[b].T tiled over channel
            ones_sb = const.tile([1, T], bf16)
            wqk_sb = const.tile([128, 6, 384], bf16)
            wqkb_sb = const.tile([1, 384], bf16)
            wv_sb = const.tile([128, 6, 195], bf16)
            wvb_sb = const.tile([1, 195], bf16)
            wp0_sb = const.tile([128, 768], bf16)
            wp1_sb = const.tile([65, 768], bf16)
            mask_sb = const.tile([128, 128], bf16)
            qk_sb = big.tile([128, 3, T], bf16)         # [Q0|Q1],[K0|K1],[Q2|K2]
            kq2_sb = big.tile([128, T], bf16)           # [K2 @0:64 | Q2 @64:128]
            v_sb = big.tile([128, NKT, 195], bf16)      # v natural, per-head 65 cols
            attu_sb = [big.tile([65, T], f32, name=f"attu{h}") for h in range(HPC)]  # unnormalized attT + Z
            attn0_sb = big.tile([128, T], bf16)         # normalized attT h0|h1
            attn1_sb = big.tile([65, T], bf16)          # h2 | ones row

            nc.vector.memset(ones_sb, 1.0)
            nc.vector.memset(attn1_sb[64:65, :], 1.0)

            # ---- loads ----
            # single-DMA multi-slab loads (in AP enumerates [128 rows, 6
            # slabs, cols] to match out dims); first 512 x-columns load
            # first so phase 1 can start early
            def slab_load(out_ap, dram_t, rowlen, ncols, col0=0):
                base = dram_t[:, :]
                src = bass.AP(
                    tensor=base.tensor,
                    offset=base.offset + col0,
                    ap=[[rowlen, 128], [128 * rowlen, 6], [1, ncols]],
                )
                nc.sync.dma_start(out=out_ap, in_=src)

            slab_load(xt_sb[:, :, 0:512], xt_d, T, 512)
            slab_load(wqk_sb, wqk_d, 384, 384)
            slab_load(wv_sb, wv_d, 195, 195)
            slab_load(xt_sb[:, :, 512:1024], xt_d, T, 512, col0=512)
            nc.sync.dma_start(out=wqkb_sb, in_=wqkb_d[:, :])
            nc.sync.dma_start(out=wvb_sb, in_=wvb_d[:, :])
            nc.sync.dma_start(out=wp0_sb, in_=wp_d[0:128, :])
            nc.sync.dma_start(out=wp1_sb, in_=wp_d[128:193, :])
            nc.sync.dma_start(out=mask_sb, in_=mask_d[:, :])
            slab_load(xt_sb[:, :, 1024:2048], xt_d, T, 1024, col0=1024)
            slab_load(xt_sb[:, :, 2048:4096], xt_d, T, 2048, col0=2048)

            # ---- phase 1a: qkT = (wqk.T @ xt) + bias   -> [384, T] bf16 ----
            with tc.tile_pool(name="psq", bufs=2, space="PSUM") as psq:
                for n in range(NQ):
                    nsl = slice(n * 512, (n + 1) * 512)
                    for m in range(3):
                        msl = slice(m * 128, (m + 1) * 128)
                        ps = psq.tile([128, 512], f32)
                        for k in range(6):
                            nc.tensor.matmul(
                                ps,
                                lhsT=wqk_sb[:, k, msl],
                                rhs=xt_sb[:, k, nsl],
                                start=(k == 0),
                                stop=False,
                            )
                        nc.tensor.matmul(
                            ps,
                            lhsT=wqkb_sb[:, msl],
                            rhs=ones_sb[:, nsl],
                            start=False,
                            stop=True,
                        )
                        if (n * 3 + m) % 2 == 0:
                            nc.scalar.copy(out=qk_sb[:, m, nsl], in_=ps)
                        else:
                            nc.vector.tensor_copy(qk_sb[:, m, nsl], ps)

            # ---- phase 1b: v natural = xt.T @ wv + bias -> [T, 195] bf16 ----
            with tc.tile_pool(name="psv", bufs=2, space="PSUM") as psv:
                for tt in range(NKT):
                    tsl = slice(tt * 128, (tt + 1) * 128)
                    ps = psv.tile([128, 195], f32)
                    for k in range(6):
                        nc.tensor.matmul(
                            ps,
                            lhsT=xt_sb[:, k, tsl],
                            rhs=wv_sb[:, k, :],
                            start=(k == 0),
                            stop=False,
                        )
                    nc.tensor.matmul(
                        ps, lhsT=ones_sb[:, tsl], rhs=wvb_sb, start=False, stop=True
                    )
                    if tt % 2 == 0:
                        nc.vector.tensor_copy(v_sb[:, tt, :], ps)
                    else:
                        nc.scalar.copy(out=v_sb[:, tt, :], in_=ps)

            # h2 duplicates so consecutive h2 score matmuls alternate row-groups
            nc.sync.dma_start(out=kq2_sb[0:64, :], in_=qk_sb[64:128, 2, :])   # K2 -> base 0
            nc.sync.dma_start(out=kq2_sb[64:128, :], in_=qk_sb[0:64, 2, :])   # Q2 -> base 64

            def K_ap(h, kt):
                sl = slice(kt * 128, (kt + 1) * 128)
                if h == 0:
                    return qk_sb[0:64, 1, sl]
                if h == 1:
                    return qk_sb[64:128, 1, sl]
                return qk_sb[64:128, 2, sl] if kt % 2 == 0 else kq2_sb[0:64, sl]

            def Q_ap(h, qi, kt):
                sl = slice(qi * 512, (qi + 1) * 512)
                if h == 0:
                    return qk_sb[0:64, 0, sl]
                if h == 1:
                    return qk_sb[64:128, 0, sl]
                return kq2_sb[64:128, sl] if kt % 2 == 0 else qk_sb[0:64, 2, sl]

            # ---- phase 2: attention ----
            with (
                tc.tile_pool(name="sc", bufs=2, space="PSUM") as pssc,
                tc.tile_pool(name="at", bufs=1, space="PSUM") as psat,
                tc.tile_pool(name="pt", bufs=4) as ppt,
            ):
                for qi in range(NQ):
                    att = [
                        psat.tile([65, 512], f32, name=f"att{h}", tag=f"att{h}",
                                  bufs=2 if h == 0 else 1)
                        for h in range(HPC)
                    ]
                    n_kt = 4 * (qi + 1)
                    for kp in range(n_kt // 2):
                        kA, kB = 2 * kp, 2 * kp + 1
                        groups = [
                            [(0, kA), (1, kA)],
                            [(0, kB), (1, kB)],
                            [(2, kA), (2, kB)],
                        ]
                        for grp in groups:
                            sc = pssc.tile([128, 1024], f32)
                            for idx, (h, kt) in enumerate(grp):
                                nc.tensor.matmul(
                                    sc[:, idx * 512:(idx + 1) * 512],
                                    lhsT=K_ap(h, kt),
                                    rhs=Q_ap(h, qi, kt),
                                    start=True,
                                    stop=True,
                                )
                            pt = ppt.tile([128, 1024], bf16)
                            nc.scalar.activation(pt, sc, Exp, scale=1.0 / math.sqrt(DH))
                            for idx, (h, kt) in enumerate(grp):
                                j = kt - 4 * qi
                                if j >= 0:  # diagonal tile: band mask + restricted read
                                    band = pt[:, idx * 512 + 128 * j: idx * 512 + 128 * (j + 1)]
                                    nc.vector.tensor_tensor(band, band, mask_sb, mult)
                                    rhs = pt[:, idx * 512 + 128 * j:(idx + 1) * 512]
                                    outap = att[h][:, 128 * j:512]
                                else:
                                    rhs = pt[:, idx * 512:(idx + 1) * 512]
                                    outap = att[h][:, :]
                                nc.tensor.matmul(
                                    outap,
                                    lhsT=v_sb[:, kt, h * 65:(h + 1) * 65],
                                    rhs=rhs,
                                    start=(kt == 0),
                                    stop=(kt == n_kt - 1),
                                    skip_group_check=True,
                                )
                    for h in range(HPC):
                        nc.vector.tensor_copy(
                            attu_sb[h][:, qi * 512:(qi + 1) * 512], att[h]
                        )

            # ---- phase 3: normalize  attT_norm = attT_unnorm * (1/Z) ----
            with (
                tc.tile_pool(name="zdram", bufs=2, space="DRAM") as zdram,
                tc.tile_pool(name="zsmall", bufs=4) as zsmall,
                tc.tile_pool(name="rbp", bufs=1) as rbp,
            ):
                for h in range(HPC):
                    zbuf = zdram.tile([1, T], f32)
                    rbuf = zdram.tile([1, T], f32)
                    nc.sync.dma_start(out=zbuf[:, :], in_=attu_sb[h][64:65, :])
                    zt = zsmall.tile([128, T // 128], f32)
                    nc.sync.dma_start(
                        out=zt, in_=zbuf.rearrange("p (a b) -> (p a) b", a=128)
                    )
                    rz = zsmall.tile([128, T // 128], f32)
                    nc.vector.reciprocal(rz, zt)
                    nc.sync.dma_start(
                        out=rbuf.rearrange("p (a b) -> (p a) b", a=128), in_=rz
                    )
                    rb = rbp.tile([64, T], f32)
                    nc.sync.dma_start(
                        out=rb,
                        in_=bass.AP(tensor=rbuf.tensor, offset=rbuf.offset, ap=[[0, 64], [1, T]]),
                    )
                    dst = (
                        attn0_sb[0:64, :]
                        if h == 0
                        else (attn0_sb[64:128, :] if h == 1 else attn1_sb[0:64, :])
                    )
                    nc.vector.tensor_tensor(dst, attu_sb[h][0:64, :], rb, mult)

            # ---- phase 4: y^T partial = wp.T @ attT_norm (+ b/4 via ones row) ----
            with (
                tc.tile_pool(name="psp", bufs=3, space="PSUM") as psp,
                tc.tile_pool(name="yst", bufs=4) as yst,
            ):
                for n in range(NQ):
                    nsl = slice(n * 512, (n + 1) * 512)
                    for m in range(6):
                        msl = slice(m * 128, (m + 1) * 128)
                        ps = psp.tile([128, 512], f32)
                        nc.tensor.matmul(
                            ps, lhsT=wp0_sb[:, msl], rhs=attn0_sb[:, nsl],
                            start=True, stop=False,
                        )
                        nc.tensor.matmul(
                            ps, lhsT=wp1_sb[:, msl], rhs=attn1_sb[:, nsl],
                            start=False, stop=True,
                        )
                        ysb = yst.tile([128, 512], bf16)
                        if m % 2 == 0:
                            nc.vector.tensor_copy(ysb, ps)
                        else:
                            nc.scalar.copy(out=ysb, in_=ps)
                        nc.sync.dma_start(out=yt_d[msl, nsl], in_=ysb)

    nc.finalize()
    return nc


def _get_program():
    global _PROG
    if _PROG is None:
        _PROG = _build_program()
    return _PROG


def _core_inputs(x, W_attn, b_attn, W_proj, b_proj, core):
    b = core // 4
    h0 = HPC * (core % 4)

    def qcol(h):
        return W_attn[:, h * 64:(h + 1) * 64]

    def kcol(h):
        return W_attn[:, C + h * 64:C + (h + 1) * 64]

    def vcol(h):
        return W_attn[:, 2 * C + h * 64:2 * C + (h + 1) * 64]

    def qb(h):
        return b_attn[h * 64:(h + 1) * 64]

    def kb(h):
        return b_attn[C + h * 64:C + (h + 1) * 64]

    def vb(h):
        return b_attn[2 * C + h * 64:2 * C + (h + 1) * 64]

    xt = np.ascontiguousarray(x[b].T).astype(BF16)
    wqk = np.concatenate(
        [qcol(h0), qcol(h0 + 1), kcol(h0), kcol(h0 + 1), qcol(h0 + 2), kcol(h0 + 2)],
        axis=1,
    ).astype(BF16)
    wqkb = np.concatenate(
        [qb(h0), qb(h0 + 1), kb(h0), kb(h0 + 1), qb(h0 + 2), kb(h0 + 2)]
    )[None, :].astype(BF16)
    zcol = np.zeros((C, 1), np.float32)
    wv = np.concatenate(
        [vcol(h0), zcol, vcol(h0 + 1), zcol, vcol(h0 + 2), zcol], axis=1
    ).astype(BF16)
    one = np.ones((1,), np.float32)
    wvb = np.concatenate([vb(h0), one, vb(h0 + 1), one, vb(h0 + 2), one])[None, :].astype(BF16)
    wp = np.concatenate(
        [W_proj[h0 * 64:(h0 + HPC) * 64, :], (b_proj / 4.0)[None, :]], axis=0
    ).astype(BF16)
    mask = np.triu(np.ones((128, 128), np.float32)).astype(BF16)
    return {
        "xt": xt, "wqk": wqk, "wqkb": wqkb, "wv": wv, "wvb": wvb,
        "wp": wp, "mask": mask,
    }


def _run(x, W_attn, b_attn, W_proj, b_proj, trace=False):
    from concourse.bass_utils import run_bass_kernel_spmd

    x = np.asarray(x, dtype=np.float32)
    W_attn = np.asarray(W_attn, dtype=np.float32)
    b_attn = np.asarray(b_attn, dtype=np.float32)
    W_proj = np.asarray(W_proj, dtype=np.float32)
    b_proj = np.asarray(b_proj, dtype=np.float32)

    nc = _get_program()
    in_maps = [
        _core_inputs(x, W_attn, b_attn, W_proj, b_proj, c) for c in range(NCORES)
    ]
    res = run_bass_kernel_spmd(
        nc, in_maps, core_ids=list(range(NCORES)), trace=trace
    )
    y = np.zeros((B, T, C), np.float32)
    for c in range(NCORES):
        y[c // 4] += res.results[c]["yt"].astype(np.float32).T
    return y, res


def kernel(x, W_attn, b_attn, W_proj, b_proj):
    y, _ = _run(x, W_attn, b_attn, W_proj, b_proj)
    return y


# revision 26
# speedup vs baseline: 1.1033x; 1.0268x over previous
"""Causal self-attention (GPT-style) Trainium2 Bass kernel.

Problem: x[2,4096,768] -> qkv = x@W_attn+b_attn -> 12-head causal attention
-> out @ W_proj + b_proj.   B=2, T=4096, C=768, H=12, Dh=64.

Sharding: (batch, head) parallel over 8 cores. Core c handles batch c//4 and
heads 3*(c%4) .. 3*(c%4)+2.  Each core computes qkv for its 3 heads, causal
attention, and a partial output projection (rows of W_proj for its heads).
Host sums the 4 partials per batch (tensor-parallel reduce) and transposes.

On-device layout choices:
- Scores are computed transposed: S^T[t_k, q] = K^T_tile.T @ Q^T so the
  exp output P^T feeds the PV matmul directly (no on-chip transposes).
- Softmax denominator Z comes for free from an appended ones-column on V
  (PV matmul row 64 = column sums of P^T).
- Softmax max-subtraction is skipped: scores are ~N(0,1) for this problem
  (max |s| < ~8), far from fp32/bf16 overflow.
- Matmul inputs are bf16 (host-precast), accumulation fp32.
- Causal masking: only lower-left k-tiles are computed; the 4 diagonal
  k-tiles per q-chunk get a 128x128 triangular band mask multiplied into
  P^T on GPSIMD, plus a restricted PV read range.
"""

import math
import sys

sys.path.insert(0, "/opt/trn_rl_repo")

import numpy as np
import ml_dtypes

B, T, C = 2, 4096, 768
NH, DH = 12, 64
HPC = 3          # heads per core
NCORES = 8
NQ = T // 512    # q chunks (8)
NKT = T // 128   # k tiles (32)

BF16 = ml_dtypes.bfloat16

_PROG = None


def _build_program():
    import concourse.bass as bass
    import concourse.mybir as mybir
    import concourse.tile as tile
    from concourse import bacc

    f32 = mybir.dt.float32
    bf16 = mybir.dt.bfloat16
    Exp = mybir.ActivationFunctionType.Exp
    mult = mybir.AluOpType.mult

    nc = bacc.Bacc("TRN2", target_bir_lowering=False)

    # ---- I/O ----
    xt_d = nc.dram_tensor("xt", [C, T], bf16, kind="ExternalInput")       # x[b].T
    wqk_d = nc.dram_tensor("wqk", [C, 384], bf16, kind="ExternalInput")   # [Q0|Q1|K0|K1|Q2|K2]
    wqkb_d = nc.dram_tensor("wqkb", [1, 384], bf16, kind="ExternalInput")
    wv_d = nc.dram_tensor("wv", [C, 195], bf16, kind="ExternalInput")     # per head [Wv|0]
    wvb_d = nc.dram_tensor("wvb", [1, 195], bf16, kind="ExternalInput")   # per head [b_v|1]
    wp_d = nc.dram_tensor("wp", [193, 768], bf16, kind="ExternalInput")   # rows + bias/4 row
    mask_d = nc.dram_tensor("mask", [128, 128], bf16, kind="ExternalInput")  # triu ones
    yt_d = nc.dram_tensor("yt", [C, T], bf16, kind="ExternalOutput")      # y[b].T partial

    with tile.TileContext(nc) as tc:
        with (
            tc.tile_pool(name="const", bufs=1) as const,
            tc.tile_pool(name="big", bufs=1) as big,
        ):
            # persistent SBUF tensors
            xt_sb = big.tile([128, 6, T], bf16)         # x[b].T tiled over channel
            ones_sb = const.tile([1, T], bf16)
            wqk_sb = const.tile([128, 6, 384], bf16)
            wqkb_sb = const.tile([1, 384], bf16)
            wv_sb = const.tile([128, 6, 195], bf16)
            wvb_sb = const.tile([1, 195], bf16)
            wp0_sb = const.tile([128, 768], bf16)
            wp1_sb = const.tile([65, 768], bf16)
            mask_sb = const.tile([128, 128], bf16)
            qk_sb = big.tile([128, 3, T], bf16)         # [Q0|Q1],[K0|K1],[Q2|K2]
            kq2_sb = big.tile([128, T], bf16)           # [K2 @0:64 | Q2 @64:128]
            v_sb = big.tile([128, NKT, 195], bf16)      # v natural, per-head 65 cols
            attu_sb = [big.tile([65, T], f32, name=f"attu{h}") for h in range(HPC)]  # unnormalized attT + Z
            attn0_sb = big.tile([128, T], bf16)         # normalized attT h0|h1
            attn1_sb = big.tile([65, T], bf16)          # h2 | ones row

            nc.vector.memset(ones_sb, 1.0)
            nc.vector.memset(attn1_sb[64:65, :], 1.0)

            # ---- loads ----
            # single-DMA multi-slab loads (in AP enumerates [128 rows, 6
            # slabs, cols] to match out dims); first 512 x-columns load
            # first so phase 1 can start early
            def slab_load(out_ap, dram_t, rowlen, ncols, col0=0):
                base = dram_t[:, :]
                src = bass.AP(
                    tensor=base.tensor,
                    offset=base.offset + col0,
                    ap=[[rowlen, 128], [128 * rowlen, 6], [1, ncols]],
                )
                nc.sync.dma_start(out=out_ap, in_=src)

            slab_load(xt_sb[:, :, 0:512], xt_d, T, 512)
            slab_load(wqk_sb, wqk_d, 384, 384)
            slab_load(wv_sb, wv_d, 195, 195)
            slab_load(xt_sb[:, :, 512:1024], xt_d, T, 512, col0=512)
            nc.sync.dma_start(out=wqkb_sb, in_=wqkb_d[:, :])
            nc.sync.dma_start(out=wvb_sb, in_=wvb_d[:, :])
            nc.sync.dma_start(out=wp0_sb, in_=wp_d[0:128, :])
            nc.sync.dma_start(out=wp1_sb, in_=wp_d[128:193, :])
            nc.sync.dma_start(out=mask_sb, in_=mask_d[:, :])
            slab_load(xt_sb[:, :, 1024:2048], xt_d, T, 1024, col0=1024)
            slab_load(xt_sb[:, :, 2048:4096], xt_d, T, 2048, col0=2048)

            # ---- phase 1a: qkT = (wqk.T @ xt) + bias   -> [384, T] bf16 ----
            with tc.tile_pool(name="psq", bufs=2, space="PSUM") as psq:
                for n in range(NQ):
                    nsl = slice(n * 512, (n + 1) * 512)
                    for m in range(3):
                        msl = slice(m * 128, (m + 1) * 128)
                        ps = psq.tile([128, 512], f32)
                        for k in range(6):
                            nc.tensor.matmul(
                                ps,
                                lhsT=wqk_sb[:, k, msl],
                                rhs=xt_sb[:, k, nsl],
                                start=(k == 0),
                                stop=False,
                            )
                        nc.tensor.matmul(
                            ps,
                            lhsT=wqkb_sb[:, msl],
                            rhs=ones_sb[:, nsl],
                            start=False,
                            stop=True,
                        )
                        if (n * 3 + m) % 2 == 0:
                            nc.scalar.copy(out=qk_sb[:, m, nsl], in_=ps)
                        else:
                            nc.vector.tensor_copy(qk_sb[:, m, nsl], ps)

            # h2 duplicates emitted here so the DMAs overlap phase 1b
            nc.sync.dma_start(out=kq2_sb[0:64, :], in_=qk_sb[64:128, 2, :])   # K2 -> base 0
            nc.sync.dma_start(out=kq2_sb[64:128, :], in_=qk_sb[0:64, 2, :])   # Q2 -> base 64

            # ---- phase 1b: v natural = xt.T @ wv + bias -> [T, 195] bf16 ----
            with tc.tile_pool(name="psv", bufs=2, space="PSUM") as psv:
                for tt in range(NKT):
                    tsl = slice(tt * 128, (tt + 1) * 128)
                    ps = psv.tile([128, 195], f32)
                    for k in range(6):
                        nc.tensor.matmul(
                            ps,
                            lhsT=xt_sb[:, k, tsl],
                            rhs=wv_sb[:, k, :],
                            start=(k == 0),
                            stop=False,
                        )
                    nc.tensor.matmul(
                        ps, lhsT=ones_sb[:, tsl], rhs=wvb_sb, start=False, stop=True
                    )
                    if tt % 2 == 0:
                        nc.vector.tensor_copy(v_sb[:, tt, :], ps)
                    else:
                        nc.scalar.copy(out=v_sb[:, tt, :], in_=ps)

            def K_ap(h, kt):
                sl = slice(kt * 128, (kt + 1) * 128)
                if h == 0:
                    return qk_sb[0:64, 1, sl]
                if h == 1:
                    return qk_sb[64:128, 1, sl]
                return qk_sb[64:128, 2, sl] if kt % 2 == 0 else kq2_sb[0:64, sl]

            def Q_ap(h, qi, kt):
                sl = slice(qi * 512, (qi + 1) * 512)
                if h == 0:
                    return qk_sb[0:64, 0, sl]
                if h == 1:
                    return qk_sb[64:128, 0, sl]
                return kq2_sb[64:128, sl] if kt % 2 == 0 else qk_sb[0:64, 2, sl]

            # ---- phase 2: attention ----
            with (
                tc.tile_pool(name="sc", bufs=2, space="PSUM") as pssc,
                tc.tile_pool(name="at", bufs=1, space="PSUM") as psat,
                tc.tile_pool(name="pt", bufs=4) as ppt,
            ):
                for qi in range(NQ):
                    att = [
                        psat.tile([65, 512], f32, name=f"att{h}", tag=f"att{h}",
                                  bufs=2 if h == 0 else 1)
                        for h in range(HPC)
                    ]
                    n_kt = 4 * (qi + 1)
                    for kp in range(n_kt // 2):
                        kA, kB = 2 * kp, 2 * kp + 1
                        groups = [
                            [(0, kA), (1, kA)],
                            [(0, kB), (1, kB)],
                            [(2, kA), (2, kB)],
                        ]
                        for grp in groups:
                            sc = pssc.tile([128, 1024], f32)
                            for idx, (h, kt) in enumerate(grp):
                                nc.tensor.matmul(
                                    sc[:, idx * 512:(idx + 1) * 512],
                                    lhsT=K_ap(h, kt),
                                    rhs=Q_ap(h, qi, kt),
                                    start=True,
                                    stop=True,
                                )
                            pt = ppt.tile([128, 1024], bf16)
                            nc.scalar.activation(pt, sc, Exp, scale=1.0 / math.sqrt(DH))
                            for idx, (h, kt) in enumerate(grp):
                                j = kt - 4 * qi
                                if j >= 0:  # diagonal tile: band mask + restricted read
                                    band = pt[:, idx * 512 + 128 * j: idx * 512 + 128 * (j + 1)]
                                    nc.vector.tensor_tensor(band, band, mask_sb, mult)
                                    rhs = pt[:, idx * 512 + 128 * j:(idx + 1) * 512]
                                    outap = att[h][:, 128 * j:512]
                                else:
                                    rhs = pt[:, idx * 512:(idx + 1) * 512]
                                    outap = att[h][:, :]
                                nc.tensor.matmul(
                                    outap,
                                    lhsT=v_sb[:, kt, h * 65:(h + 1) * 65],
                                    rhs=rhs,
                                    start=(kt == 0),
                                    stop=(kt == n_kt - 1),
                                    skip_group_check=True,
                                )
                    for h in range(HPC):
                        nc.vector.tensor_copy(
                            attu_sb[h][:, qi * 512:(qi + 1) * 512], att[h]
                        )

            # ---- phase 3: normalize  attT_norm = attT_unnorm * (1/Z) ----
            with (
                tc.tile_pool(name="zdram", bufs=2, space="DRAM") as zdram,
                tc.tile_pool(name="zsmall", bufs=4) as zsmall,
                tc.tile_pool(name="rbp", bufs=1) as rbp,
            ):
                for h in range(HPC):
                    zbuf = zdram.tile([1, T], f32)
                    rbuf = zdram.tile([1, T], bf16)
                    nc.sync.dma_start(out=zbuf[:, :], in_=attu_sb[h][64:65, :])
                    zt = zsmall.tile([128, T // 128], f32)
                    nc.sync.dma_start(
                        out=zt, in_=zbuf.rearrange("p (a b) -> (p a) b", a=128)
                    )
                    rz = zsmall.tile([128, T // 128], bf16)
                    with nc.allow_low_precision(reason="1/Z broadcast in bf16"):
                        nc.vector.reciprocal(rz, zt)
                    nc.sync.dma_start(
                        out=rbuf.rearrange("p (a b) -> (p a) b", a=128), in_=rz
                    )
                    rb = rbp.tile([64, T], bf16)
                    nc.sync.dma_start(
                        out=rb,
                        in_=bass.AP(tensor=rbuf.tensor, offset=rbuf.offset, ap=[[0, 64], [1, T]]),
                    )
                    dst = (
                        attn0_sb[0:64, :]
                        if h == 0
                        else (attn0_sb[64:128, :] if h == 1 else attn1_sb[0:64, :])
                    )
                    nc.vector.tensor_tensor(dst, attu_sb[h][0:64, :], rb, mult)

            # ---- phase 4: y^T partial = wp.T @ attT_norm (+ b/4 via ones row) ----
            with (
                tc.tile_pool(name="psp", bufs=3, space="PSUM") as psp,
                tc.tile_pool(name="yst", bufs=4) as yst,
            ):
                for n in range(NQ):
                    nsl = slice(n * 512, (n + 1) * 512)
                    for m in range(6):
                        msl = slice(m * 128, (m + 1) * 128)
                        ps = psp.tile([128, 512], f32)
                        nc.tensor.matmul(
                            ps, lhsT=wp0_sb[:, msl], rhs=attn0_sb[:, nsl],
                            start=True, stop=False,
                        )
                        nc.tensor.matmul(
                            ps, lhsT=wp1_sb[:, msl], rhs=attn1_sb[:, nsl],
                            start=False, stop=True,
                        )
                        ysb = yst.tile([128, 512], bf16)
                        if m % 2 == 0:
                            nc.vector.tensor_copy(ysb, ps)
                        else:
                            nc.scalar.copy(out=ysb, in_=ps)
                        nc.sync.dma_start(out=yt_d[msl, nsl], in_=ysb)

    nc.finalize()
    return nc


def _get_program():
    global _PROG
    if _PROG is None:
        _PROG = _build_program()
    return _PROG


def _core_inputs(x, W_attn, b_attn, W_proj, b_proj, core):
    b = core // 4
    h0 = HPC * (core % 4)

    def qcol(h):
        return W_attn[:, h * 64:(h + 1) * 64]

    def kcol(h):
        return W_attn[:, C + h * 64:C + (h + 1) * 64]

    def vcol(h):
        return W_attn[:, 2 * C + h * 64:2 * C + (h + 1) * 64]

    def qb(h):
        return b_attn[h * 64:(h + 1) * 64]

    def kb(h):
        return b_attn[C + h * 64:C + (h + 1) * 64]

    def vb(h):
        return b_attn[2 * C + h * 64:2 * C + (h + 1) * 64]

    xt = np.ascontiguousarray(x[b].T).astype(BF16)
    wqk = np.concatenate(
        [qcol(h0), qcol(h0 + 1), kcol(h0), kcol(h0 + 1), qcol(h0 + 2), kcol(h0 + 2)],
        axis=1,
    ).astype(BF16)
    wqkb = np.concatenate(
        [qb(h0), qb(h0 + 1), kb(h0), kb(h0 + 1), qb(h0 + 2), kb(h0 + 2)]
    )[None, :].astype(BF16)
    zcol = np.zeros((C, 1), np.float32)
    wv = np.concatenate(
        [vcol(h0), zcol, vcol(h0 + 1), zcol, vcol(h0 + 2), zcol], axis=1
    ).astype(BF16)
    one = np.ones((1,), np.float32)
    wvb = np.concatenate([vb(h0), one, vb(h0 + 1), one, vb(h0 + 2), one])[None, :].astype(BF16)
    wp = np.concatenate(
        [W_proj[h0 * 64:(h0 + HPC) * 64, :], (b_proj / 4.0)[None, :]], axis=0
    ).astype(BF16)
    mask = np.triu(np.ones((128, 128), np.float32)).astype(BF16)
    return {
        "xt": xt, "wqk": wqk, "wqkb": wqkb, "wv": wv, "wvb": wvb,
        "wp": wp, "mask": mask,
    }


def _run(x, W_attn, b_attn, W_proj, b_proj, trace=False):
    from concourse.bass_utils import run_bass_kernel_spmd

    x = np.asarray(x, dtype=np.float32)
    W_attn = np.asarray(W_attn, dtype=np.float32)
    b_attn = np.asarray(b_attn, dtype=np.float32)
    W_proj = np.asarray(W_proj, dtype=np.float32)
    b_proj = np.asarray(b_proj, dtype=np.float32)

    nc = _get_program()
    in_maps = [
        _core_inputs(x, W_attn, b_attn, W_proj, b_proj, c) for c in range(NCORES)
    ]
    res = run_bass_kernel_spmd(
        nc, in_maps, core_ids=list(range(NCORES)), trace=trace
    )
    y = np.zeros((B, T, C), np.float32)
    for c in range(NCORES):
        y[c // 4] += res.results[c]["yt"].astype(np.float32).T
    return y, res


def kernel(x, W_attn, b_attn, W_proj, b_proj):
    y, _ = _run(x, W_attn, b_attn, W_proj, b_proj)
    return y
